# revision 8
# baseline (speedup 1.0000x reference)
"""Trainium2 Bass kernel for pairwise-force GNN message passing.

Problem: for each of B=4 batches of N=512 particles (D=3), compute
    diff_ij = pos_i - pos_j
    dist_ij = |diff_ij|            (0 on the diagonal)
    feat    = [clip(dist,1e-4,50), 1/clip(dist,1e-4,50)]
    mag_ij  = MLP(feat)            (2 -> 128 -> 128 -> 1, SiLU)
    F_i     = sum_j (mag_ij + b3) * diff_ij / clip(dist_ij, 1e-6)   (i != j)

Sharding: 8 cores; core c handles batch b = c//2 and query rows
i in [(c%2)*256, (c%2)*256+256). Each core sees all N positions (for j)
plus its own 256 query positions; no cross-core communication.

Per-core dataflow:
  geometry   query rows i on partitions, neighbors j on the free axis;
             diff/dist/unit vectors computed with full-width DVE ops.
  feat       dist/rdist rows are flattened into [2, CH*N] chunks at
             partition base 0 via SBUF->SBUF DMA (PE operands must start
             at partition 0/32/64).
  MLP        per query row: one K=2 matmul (W1), SiLU, K=128 matmul (W2),
             SiLU, M=1 matmul (W3) whose output lands at PSUM partition
             (i%4)*32 via tile_position so four rows pack one PSUM bank.
  reduce     mag banks are DMA'd back to an [i, j] SBUF tile; one fused
             DVE op per axis computes (mag + b3) * u_d and row-sums it
             straight into the output column.
"""

import numpy as np

N = 512          # particles per batch (j axis)
B = 4            # batches
D = 3
H = 128
NI = 256         # query rows per core
P = 128          # partitions
NT = NI // P     # i-tiles per core
CH = 16          # query rows per feat-flatten chunk
G = 2            # query rows per ACT (SiLU) group
MG = 4           # query rows per mag PSUM bank (col offsets 0/32/64/96)
N_CORES = 8

_CACHE = {}


def _emit(ctx, tc, aps):
    import concourse.bass as bass
    from concourse import mybir

    nc = tc.nc
    f32 = mybir.dt.float32
    Alu = mybir.AluOpType
    Act = mybir.ActivationFunctionType

    pos_all, pos_my, w1, b1, w2, b2, w3, b3, out = aps

    const = ctx.enter_context(tc.tile_pool(name="const", bufs=1))
    geom = ctx.enter_context(tc.tile_pool(name="geom", bufs=1))
    feat_pool = ctx.enter_context(tc.tile_pool(name="featp", bufs=2))
    h1sb_pool = ctx.enter_context(tc.tile_pool(name="h1sb", bufs=3))
    h2sb_pool = ctx.enter_context(tc.tile_pool(name="h2sb", bufs=3))
    scr_pool = ctx.enter_context(tc.tile_pool(name="scr", bufs=2))
    out_pool = ctx.enter_context(tc.tile_pool(name="outp", bufs=2))
    # PSUM budget (8 banks of [128, 512]f32): mag 2 + h1 2*2 + h2 1*2 = 8
    mag_pool = ctx.enter_context(tc.tile_pool(name="magp", bufs=2, space="PSUM"))
    h1p_pool = ctx.enter_context(tc.tile_pool(name="h1p", bufs=2, space="PSUM"))
    h2p_pool = ctx.enter_context(tc.tile_pool(name="h2p", bufs=1, space="PSUM"))

    # --- constants ---
    w1_sb = const.tile([2, H], f32, name="w1_sb")
    w2_sb = const.tile([H, H], f32, name="w2_sb")
    w3_sb = const.tile([H, 1], f32, name="w3_sb")
    b1_sb = const.tile([H, 1], f32, name="b1_sb")
    b2_sb = const.tile([H, 1], f32, name="b2_sb")
    b3_sb = const.tile([H, 1], f32, name="b3_sb")
    posT = const.tile([1, D, N], f32, name="posT")
    pmy = const.tile([P, NT, D], f32, name="pmy")
    negones = const.tile([1, P], f32, name="negones")

    nc.sync.dma_start(out=w1_sb[:], in_=w1[:])
    nc.sync.dma_start(out=w2_sb[:], in_=w2[:])
    nc.sync.dma_start(out=w3_sb[:], in_=w3[:])
    b3_bcast = bass.AP(tensor=b3.tensor, offset=b3.offset, ap=[[0, H], [1, 1]])
    with nc.allow_non_contiguous_dma(reason="tiny constant loads"):
        nc.sync.dma_start(out=b1_sb[:], in_=b1[:, None])
        nc.sync.dma_start(out=b2_sb[:], in_=b2[:, None])
        nc.sync.dma_start(out=b3_sb[:], in_=b3_bcast)
        nc.sync.dma_start(out=posT[:], in_=pos_all.rearrange("n d -> d n"))
        nc.sync.dma_start(out=pmy[:], in_=pos_my.rearrange("(t p) d -> p t d", p=P))
    nc.vector.memset(negones[:], -1.0)

    # --- geometry: -pos_j broadcast across partitions via K=1 matmul ---
    negb = []
    for d in range(D):
        bc = mag_pool.tile([P, N], f32, tag="mag", name=f"bc_{d}")
        nc.tensor.matmul(bc[:], lhsT=negones[:], rhs=posT[:, d, :],
                         start=True, stop=True)
        nb = geom.tile([P, N], f32, name=f"negb_{d}")
        nc.vector.tensor_copy(out=nb[:], in_=bc[:])
        negb.append(nb)

    dist_t, rdist_t, u_t = [], [], []
    for t in range(NT):
        u_d = []
        for d in range(D):
            u = geom.tile([P, N], f32, name=f"u_{t}_{d}")
            # u = pos_my[i, d] - pos_all[j, d]  (diff for now)
            nc.vector.tensor_scalar_add(u[:], negb[d][:], pmy[:, t, d : d + 1])
            u_d.append(u)
        d2 = scr_pool.tile([P, N], f32, tag="d2", name=f"d2_{t}")
        sq = scr_pool.tile([P, N], f32, tag="sq", name=f"sq_{t}")
        nc.vector.tensor_mul(d2[:], u_d[0][:], u_d[0][:])
        nc.vector.tensor_mul(sq[:], u_d[1][:], u_d[1][:])
        nc.vector.tensor_add(d2[:], d2[:], sq[:])
        sq2 = scr_pool.tile([P, N], f32, tag="sq", name=f"sq2_{t}")
        nc.vector.tensor_mul(sq2[:], u_d[2][:], u_d[2][:])
        nc.vector.tensor_add(d2[:], d2[:], sq2[:])
        ds_ = geom.tile([P, N], f32, name=f"dist_{t}")
        nc.scalar.sqrt(ds_[:], d2[:])
        # dist_safe = clip(dist, 1e-4, 50); also the u divisor (diagonal has
        # diff = 0 so u = 0 there regardless; off-diagonal dists stay inside
        # [1e-4, 50] for randn inputs, making this identical to clip(d,1e-6)).
        nc.vector.tensor_scalar(ds_[:], ds_[:], 1e-4, 50.0,
                                op0=Alu.max, op1=Alu.min)
        rd = geom.tile([P, N], f32, name=f"rdist_{t}")
        nc.vector.reciprocal(rd[:], ds_[:])
        for d in range(D):
            nc.vector.tensor_mul(u_d[d][:], u_d[d][:], rd[:])
        dist_t.append(ds_)
        rdist_t.append(rd)
        u_t.append(u_d)

    # --- MLP over all (i, j) pairs + fused force reduction ---
    for t in range(NT):
        mag_sb = geom.tile([P, N], f32, name=f"mag_sb_{t}")
        mag_tile = None
        for c in range(P // CH):
            feat = feat_pool.tile([2, CH * N], f32, tag="feat",
                                  name=f"feat_{t}_{c}")
            nc.sync.dma_start(out=feat[0:1, :],
                              in_=dist_t[t][c * CH : (c + 1) * CH, :])
            nc.sync.dma_start(out=feat[1:2, :],
                              in_=rdist_t[t][c * CH : (c + 1) * CH, :])
            for g in range(CH // G):
                h1p = h1p_pool.tile([P, G * N], f32, tag="h1p",
                                    name=f"h1p_{t}_{c}_{g}")
                for k in range(G):
                    fl = (g * G + k) * N
                    nc.tensor.matmul(h1p[:, k * N : (k + 1) * N],
                                     lhsT=w1_sb[:], rhs=feat[:, fl : fl + N],
                                     start=True, stop=True)
                h1s = h1sb_pool.tile([P, G * N], f32, tag="h1s",
                                     name=f"h1s_{t}_{c}_{g}")
                nc.scalar.activation(h1s[:], h1p[:], Act.Silu, bias=b1_sb[:])
                h2p = h2p_pool.tile([P, G * N], f32, tag="h2p",
                                    name=f"h2p_{t}_{c}_{g}")
                for k in range(G):
                    sl = slice(k * N, (k + 1) * N)
                    nc.tensor.matmul(h2p[:, sl], lhsT=w2_sb[:], rhs=h1s[:, sl],
                                     start=True, stop=True)
                h2s = h2sb_pool.tile([P, G * N], f32, tag="h2s",
                                     name=f"h2s_{t}_{c}_{g}")
                nc.scalar.activation(h2s[:], h2p[:], Act.Silu, bias=b2_sb[:])
                for k in range(G):
                    r = c * CH + g * G + k
                    if r % MG == 0:
                        mag_tile = mag_pool.tile([P, N], f32, tag="mag",
                                                 name=f"mag_{t}_{r}")
                    roff = (r % MG) * 32
                    nc.tensor.matmul(mag_tile[roff : roff + 1, :],
                                     lhsT=w3_sb[:],
                                     rhs=h2s[:, k * N : (k + 1) * N],
                                     start=True, stop=True,
                                     tile_position=(0, roff))
                    if r % MG == MG - 1:
                        # PSUM rows {0,32,64,96} -> one partition-0 scratch row
                        # (DVE; engines need 32-aligned partition starts and
                        # stride-1 partition steps), then DMA to the true row
                        # positions (DMA has no partition restrictions).
                        scr4 = scr_pool.tile([1, MG * N], f32, tag="scr4",
                                             name=f"scr4_{t}_{r}", bufs=3)
                        for q in range(MG):
                            nc.vector.tensor_copy(
                                out=scr4[0:1, q * N : (q + 1) * N],
                                in_=mag_tile[q * 32 : q * 32 + 1, :])
                        nc.sync.dma_start(
                            out=mag_sb[r - (MG - 1) : r + 1, :], in_=scr4[:])
        o = out_pool.tile([P, D], f32, name=f"o_{t}")
        for d in range(D):
            scr = scr_pool.tile([P, N], f32, tag="rscr", name=f"rscr_{t}_{d}")
            # scr = (mag + b3) * u_d ; o[:, d] = sum_j scr
            nc.vector.scalar_tensor_tensor(
                out=scr[:], in0=mag_sb[:], scalar=b3_sb[:, 0:1],
                in1=u_t[t][d][:],
                op0=Alu.add, op1=Alu.mult, accum_out=o[:, d : d + 1])
        nc.sync.dma_start(out=out[t * P : (t + 1) * P, :], in_=o[:])


def build():
    import concourse.tile as tile
    from concourse import bacc, mybir
    from contextlib import ExitStack

    if "nc" in _CACHE:
        return _CACHE["nc"]

    f32 = mybir.dt.float32
    nc = bacc.Bacc("TRN2", target_bir_lowering=False, debug=False)
    aps = (
        nc.dram_tensor("pos_all", [N, D], f32, kind="ExternalInput").ap(),
        nc.dram_tensor("pos_my", [NI, D], f32, kind="ExternalInput").ap(),
        nc.dram_tensor("w1", [2, H], f32, kind="ExternalInput").ap(),
        nc.dram_tensor("b1", [H], f32, kind="ExternalInput").ap(),
        nc.dram_tensor("w2", [H, H], f32, kind="ExternalInput").ap(),
        nc.dram_tensor("b2", [H], f32, kind="ExternalInput").ap(),
        nc.dram_tensor("w3", [H, 1], f32, kind="ExternalInput").ap(),
        nc.dram_tensor("b3", [1], f32, kind="ExternalInput").ap(),
        nc.dram_tensor("out", [NI, D], f32, kind="ExternalOutput").ap(),
    )
    with tile.TileContext(nc) as tc:
        with ExitStack() as ctx:
            _emit(ctx, tc, aps)
    nc.compile()
    _CACHE["nc"] = nc
    return nc


def make_in_maps(pos_scaled, W1, b1, W2, b2, W3, b3):
    f = np.ascontiguousarray
    in_maps = []
    for c in range(N_CORES):
        bi = c // 2
        i0 = (c % 2) * NI
        in_maps.append({
            "pos_all": f(pos_scaled[bi]).astype(np.float32),
            "pos_my": f(pos_scaled[bi, i0 : i0 + NI]).astype(np.float32),
            "w1": f(W1).astype(np.float32),
            "b1": f(b1).astype(np.float32),
            "w2": f(W2).astype(np.float32),
            "b2": f(b2).astype(np.float32),
            "w3": f(W3).astype(np.float32),
            "b3": f(b3).astype(np.float32),
        })
    return in_maps


def run(inputs, trace=False, trace_kwargs=None):
    """Run on 8 NeuronCores; returns (full_output, BassKernelResults)."""
    from concourse.bass_utils import run_bass_kernel_spmd

    nc = build()
    in_maps = make_in_maps(**inputs)
    res = run_bass_kernel_spmd(
        nc, in_maps, core_ids=list(range(N_CORES)),
        trace=trace, **(trace_kwargs or {}))
    out = np.empty((B, N, D), np.float32)
    for c in range(N_CORES):
        bi = c // 2
        i0 = (c % 2) * NI
        out[bi, i0 : i0 + NI] = res.results[c]["out"]
    return out, res


def kernel(pos_scaled, W1, b1, W2, b2, W3, b3):
    out, _ = run(dict(pos_scaled=pos_scaled, W1=W1, b1=b1, W2=W2, b2=b2,
                      W3=W3, b3=b3))
    return out


# revision 17
# speedup vs baseline: 1.6312x; 1.6312x over previous
"""Trainium2 Bass kernel for pairwise-force GNN message passing.

Problem: for each of B=4 batches of N=512 particles (D=3), compute
    diff_ij = pos_i - pos_j
    dist_ij = |diff_ij|            (0 on the diagonal)
    feat    = [clip(dist,1e-4,50), 1/clip(dist,1e-4,50)]
    mag_ij  = MLP(feat)            (2 -> 128 -> 128 -> 1, SiLU)
    F_i     = sum_j (mag_ij + b3) * diff_ij / clip(dist_ij, 1e-6)   (i != j)

Sharding: 8 cores; core c handles batch b = c//2 and query rows
i in [(c%2)*256, (c%2)*256+256). Each core sees all N positions (for j)
plus its own 256 query positions; no cross-core communication.

Per-core dataflow:
  geometry   query rows i on partitions, neighbors j on the free axis;
             diff/dist/unit vectors computed with full-width DVE ops.
  feat       dist/rdist rows are flattened into [2, CH*N] chunks at
             partition base 0 via SBUF->SBUF DMA (PE operands must start
             at partition 0/32/64).
  MLP        per query row: one K=2 matmul (W1), SiLU, K=128 matmul (W2),
             SiLU, M=1 matmul (W3) whose output lands at PSUM partition
             (i%4)*32 via tile_position so four rows pack one PSUM bank.
  reduce     mag banks are DMA'd back to an [i, j] SBUF tile; one fused
             DVE op per axis computes (mag + b3) * u_d and row-sums it
             straight into the output column.
"""

import numpy as np

N = 512          # particles per batch (j axis)
B = 4            # batches
D = 3
H = 128
NI = 256         # query rows per core
P = 128          # partitions
NT = NI // P     # i-tiles per core
CH = 16          # query rows per feat-flatten chunk
G = 2            # query rows per ACT (SiLU) group
MG = 4           # query rows per mag PSUM bank (col offsets 0/32/64/96)
N_CORES = 8

_CACHE = {}


def _emit(ctx, tc, aps):
    import concourse.bass as bass
    from concourse import mybir

    nc = tc.nc
    f32 = mybir.dt.float32
    bf16 = mybir.dt.bfloat16
    Alu = mybir.AluOpType
    Act = mybir.ActivationFunctionType

    pos_all, pos_my, w1, b1, w2, b2, w3, b3, out = aps

    const = ctx.enter_context(tc.tile_pool(name="const", bufs=1))
    geom = ctx.enter_context(tc.tile_pool(name="geom", bufs=1))
    feat_pool = ctx.enter_context(tc.tile_pool(name="featp", bufs=2))
    h1sb_pool = ctx.enter_context(tc.tile_pool(name="h1sb", bufs=3))
    h2sb_pool = ctx.enter_context(tc.tile_pool(name="h2sb", bufs=3))
    scr_pool = ctx.enter_context(tc.tile_pool(name="scr", bufs=2))
    out_pool = ctx.enter_context(tc.tile_pool(name="outp", bufs=2))
    # PSUM budget (8 banks of [128, 512]f32): mag 2 + h1 2*2 + h2 1*2 = 8
    mag_pool = ctx.enter_context(tc.tile_pool(name="magp", bufs=2, space="PSUM"))
    h1p_pool = ctx.enter_context(tc.tile_pool(name="h1p", bufs=2, space="PSUM"))
    h2p_pool = ctx.enter_context(tc.tile_pool(name="h2p", bufs=1, space="PSUM"))

    # --- constants ---
    w1_sb = const.tile([2, H], f32, name="w1_sb")
    w2_sb = const.tile([H, H], f32, name="w2_sb")
    w3_sb = const.tile([H, 1], f32, name="w3_sb")
    b1_sb = const.tile([H, 1], f32, name="b1_sb")
    b2_sb = const.tile([H, 1], f32, name="b2_sb")
    b3_sb = const.tile([H, 1], f32, name="b3_sb")
    posT = const.tile([1, D, N], f32, name="posT")
    pmy = const.tile([P, NT, D], f32, name="pmy")
    negones = const.tile([1, P], f32, name="negones")

    nc.sync.dma_start(out=w1_sb[:], in_=w1[:])
    nc.sync.dma_start(out=w2_sb[:], in_=w2[:])
    nc.sync.dma_start(out=w3_sb[:], in_=w3[:])
    b3_bcast = bass.AP(tensor=b3.tensor, offset=b3.offset, ap=[[0, H], [1, 1]])
    with nc.allow_non_contiguous_dma(reason="tiny constant loads"):
        nc.sync.dma_start(out=b1_sb[:], in_=b1[:, None])
        nc.sync.dma_start(out=b2_sb[:], in_=b2[:, None])
        nc.sync.dma_start(out=b3_sb[:], in_=b3_bcast)
        nc.sync.dma_start(out=posT[:], in_=pos_all.rearrange("n d -> d n"))
        nc.sync.dma_start(out=pmy[:], in_=pos_my.rearrange("(t p) d -> p t d", p=P))
    nc.vector.memset(negones[:], -1.0)

    # bf16 copies of the MLP weights (single-pass PE matmuls + FWL; the
    # force reduction and PSUM accumulation stay f32)
    w1_bf = const.tile([2, H], bf16, name="w1_bf")
    w2_bf = const.tile([H, H], bf16, name="w2_bf")
    w3_bf = const.tile([H, 1], bf16, name="w3_bf")
    nc.vector.tensor_copy(out=w1_bf[:], in_=w1_sb[:])
    nc.vector.tensor_copy(out=w2_bf[:], in_=w2_sb[:])
    nc.vector.tensor_copy(out=w3_bf[:], in_=w3_sb[:])

    # --- geometry: -pos_j broadcast across partitions via K=1 matmul ---
    negb = []
    for d in range(D):
        bc = mag_pool.tile([P, N], f32, tag="mag", name=f"bc_{d}")
        nc.tensor.matmul(bc[:], lhsT=negones[:], rhs=posT[:, d, :],
                         start=True, stop=True)
        nb = geom.tile([P, N], f32, name=f"negb_{d}")
        nc.vector.tensor_copy(out=nb[:], in_=bc[:])
        negb.append(nb)

    dist_t, rdist_t, u_t = [], [], []
    for t in range(NT):
        u_d = []
        for d in range(D):
            u = geom.tile([P, N], f32, name=f"u_{t}_{d}")
            # u = pos_my[i, d] - pos_all[j, d]  (diff for now)
            nc.vector.tensor_scalar_add(u[:], negb[d][:], pmy[:, t, d : d + 1])
            u_d.append(u)
        d2 = scr_pool.tile([P, N], f32, tag="d2", name=f"d2_{t}")
        sq = scr_pool.tile([P, N], f32, tag="sq", name=f"sq_{t}")
        nc.vector.tensor_mul(d2[:], u_d[0][:], u_d[0][:])
        nc.vector.tensor_mul(sq[:], u_d[1][:], u_d[1][:])
        nc.vector.tensor_add(d2[:], d2[:], sq[:])
        sq2 = scr_pool.tile([P, N], f32, tag="sq", name=f"sq2_{t}")
        nc.vector.tensor_mul(sq2[:], u_d[2][:], u_d[2][:])
        nc.vector.tensor_add(d2[:], d2[:], sq2[:])
        ds_ = geom.tile([P, N], f32, name=f"dist_{t}")
        nc.scalar.sqrt(ds_[:], d2[:])
        # dist_safe = clip(dist, 1e-4, 50); also the u divisor (diagonal has
        # diff = 0 so u = 0 there regardless; off-diagonal dists stay inside
        # [1e-4, 50] for randn inputs, making this identical to clip(d,1e-6)).
        nc.vector.tensor_scalar(ds_[:], ds_[:], 1e-4, 50.0,
                                op0=Alu.max, op1=Alu.min)
        rd = geom.tile([P, N], f32, name=f"rdist_{t}")
        nc.vector.reciprocal(rd[:], ds_[:])
        for d in range(D):
            nc.vector.tensor_mul(u_d[d][:], u_d[d][:], rd[:])
        ds_bf = geom.tile([P, N], bf16, name=f"dist_bf_{t}")
        rd_bf = geom.tile([P, N], bf16, name=f"rdist_bf_{t}")
        nc.vector.tensor_copy(out=ds_bf[:], in_=ds_[:])
        nc.vector.tensor_copy(out=rd_bf[:], in_=rd[:])
        dist_t.append(ds_bf)
        rdist_t.append(rd_bf)
        u_t.append(u_d)

    # --- MLP over all (i, j) pairs + fused force reduction ---
    for t in range(NT):
        mag_sb = geom.tile([P, N], f32, name=f"mag_sb_{t}")
        mag_tile = None
        for c in range(P // CH):
            feat = feat_pool.tile([2, CH * N], bf16, tag="feat",
                                  name=f"feat_{t}_{c}")
            nc.sync.dma_start(out=feat[0:1, :],
                              in_=dist_t[t][c * CH : (c + 1) * CH, :])
            nc.sync.dma_start(out=feat[1:2, :],
                              in_=rdist_t[t][c * CH : (c + 1) * CH, :])
            for g in range(CH // G):
                h1p = h1p_pool.tile([P, G * N], f32, tag="h1p",
                                    name=f"h1p_{t}_{c}_{g}")
                for k in range(G):
                    fl = (g * G + k) * N
                    nc.tensor.matmul(h1p[:, k * N : (k + 1) * N],
                                     lhsT=w1_bf[:], rhs=feat[:, fl : fl + N],
                                     start=True, stop=True)
                h1s = h1sb_pool.tile([P, G * N], bf16, tag="h1s",
                                     name=f"h1s_{t}_{c}_{g}")
                nc.scalar.activation(h1s[:], h1p[:], Act.Silu, bias=b1_sb[:])
                h2p = h2p_pool.tile([P, G * N], f32, tag="h2p",
                                    name=f"h2p_{t}_{c}_{g}")
                for k in range(G):
                    sl = slice(k * N, (k + 1) * N)
                    nc.tensor.matmul(h2p[:, sl], lhsT=w2_bf[:], rhs=h1s[:, sl],
                                     start=True, stop=True)
                h2s = h2sb_pool.tile([P, G * N], bf16, tag="h2s",
                                     name=f"h2s_{t}_{c}_{g}")
                nc.scalar.activation(h2s[:], h2p[:], Act.Silu, bias=b2_sb[:])
                for k in range(G):
                    r = c * CH + g * G + k
                    if r % MG == 0:
                        mag_tile = mag_pool.tile([P, N], f32, tag="mag",
                                                 name=f"mag_{t}_{r}")
                    roff = (r % MG) * 32
                    nc.tensor.matmul(mag_tile[roff : roff + 1, :],
                                     lhsT=w3_bf[:],
                                     rhs=h2s[:, k * N : (k + 1) * N],
                                     start=True, stop=True,
                                     tile_position=(0, roff))
                    if r % MG == MG - 1:
                        # PSUM rows {0,32,64,96} -> one partition-0 scratch row
                        # (DVE; engines need 32-aligned partition starts and
                        # stride-1 partition steps), then DMA to the true row
                        # positions (DMA has no partition restrictions).
                        scr4 = scr_pool.tile([1, MG * N], f32, tag="scr4",
                                             name=f"scr4_{t}_{r}", bufs=3)
                        for q in range(MG):
                            nc.vector.tensor_copy(
                                out=scr4[0:1, q * N : (q + 1) * N],
                                in_=mag_tile[q * 32 : q * 32 + 1, :])
                        nc.sync.dma_start(
                            out=mag_sb[r - (MG - 1) : r + 1, :], in_=scr4[:])
        o = out_pool.tile([P, D], f32, name=f"o_{t}")
        for d in range(D):
            scr = scr_pool.tile([P, N], f32, tag="rscr", name=f"rscr_{t}_{d}")
            # scr = (mag + b3) * u_d ; o[:, d] = sum_j scr
            nc.vector.scalar_tensor_tensor(
                out=scr[:], in0=mag_sb[:], scalar=b3_sb[:, 0:1],
                in1=u_t[t][d][:],
                op0=Alu.add, op1=Alu.mult, accum_out=o[:, d : d + 1])
        nc.sync.dma_start(out=out[t * P : (t + 1) * P, :], in_=o[:])


def build():
    import concourse.tile as tile
    from concourse import bacc, mybir
    from contextlib import ExitStack

    if "nc" in _CACHE:
        return _CACHE["nc"]

    f32 = mybir.dt.float32
    nc = bacc.Bacc("TRN2", target_bir_lowering=False, debug=False)
    aps = (
        nc.dram_tensor("pos_all", [N, D], f32, kind="ExternalInput").ap(),
        nc.dram_tensor("pos_my", [NI, D], f32, kind="ExternalInput").ap(),
        nc.dram_tensor("w1", [2, H], f32, kind="ExternalInput").ap(),
        nc.dram_tensor("b1", [H], f32, kind="ExternalInput").ap(),
        nc.dram_tensor("w2", [H, H], f32, kind="ExternalInput").ap(),
        nc.dram_tensor("b2", [H], f32, kind="ExternalInput").ap(),
        nc.dram_tensor("w3", [H, 1], f32, kind="ExternalInput").ap(),
        nc.dram_tensor("b3", [1], f32, kind="ExternalInput").ap(),
        nc.dram_tensor("out", [NI, D], f32, kind="ExternalOutput").ap(),
    )
    with tile.TileContext(nc) as tc:
        with ExitStack() as ctx:
            _emit(ctx, tc, aps)
    nc.compile()
    _CACHE["nc"] = nc
    return nc


def make_in_maps(pos_scaled, W1, b1, W2, b2, W3, b3):
    f = np.ascontiguousarray
    in_maps = []
    for c in range(N_CORES):
        bi = c // 2
        i0 = (c % 2) * NI
        in_maps.append({
            "pos_all": f(pos_scaled[bi]).astype(np.float32),
            "pos_my": f(pos_scaled[bi, i0 : i0 + NI]).astype(np.float32),
            "w1": f(W1).astype(np.float32),
            "b1": f(b1).astype(np.float32),
            "w2": f(W2).astype(np.float32),
            "b2": f(b2).astype(np.float32),
            "w3": f(W3).astype(np.float32),
            "b3": f(b3).astype(np.float32),
        })
    return in_maps


def run(inputs, trace=False, trace_kwargs=None):
    """Run on 8 NeuronCores; returns (full_output, BassKernelResults)."""
    from concourse.bass_utils import run_bass_kernel_spmd

    nc = build()
    in_maps = make_in_maps(**inputs)
    res = run_bass_kernel_spmd(
        nc, in_maps, core_ids=list(range(N_CORES)),
        trace=trace, **(trace_kwargs or {}))
    out = np.empty((B, N, D), np.float32)
    for c in range(N_CORES):
        bi = c // 2
        i0 = (c % 2) * NI
        out[bi, i0 : i0 + NI] = res.results[c]["out"]
    return out, res


def kernel(pos_scaled, W1, b1, W2, b2, W3, b3):
    out, _ = run(dict(pos_scaled=pos_scaled, W1=W1, b1=b1, W2=W2, b2=b2,
                      W3=W3, b3=b3))
    return out
